# revision 1
# baseline (speedup 1.0000x reference)
"""Single-head causal attention with ALiBi (B=4, T=4096, C=HS=64) on 8 TRN2 cores.

Math: out = softmax(mask((x Wq)(x Wk)^T * C^-0.5 + (j-i)*slope)) @ (x Wv)

The ALiBi slope for this head is 2^-0.5 ~= 0.707 per step of distance, so the
softmax weight of any key more than ~128 steps behind the query underflows to
exactly 0 in fp32 (the reference computes exp(score - rowmax) <= e^-85 there).
Each query therefore only needs keys in a 256-wide sliding window: key tiles
kt-1 ("prev") and kt ("diag") for query tile kt.  This turns O(T^2) attention
into O(T*256) with bit-identical results up to fp rounding.

Sharding: 8 cores = (batch b in 0..3) x (half h in 0..1); each core handles
2048 queries [h*2048, (h+1)*2048) of batch b and receives the 2176 x-rows
[q0-128, q0+2048) (zero-padded below row 0).

Softmax without row-max: scores are shifted per-query by (fi-64)*slope (fi =
query index within its 128-tile), which cancels in the normalization but keeps
every exponent in [-88, 52] -> exp() stays in fp32/bf16 normal range.  The
shifted ALiBi bias then depends only on the key's partition index pj:
  diag tile: (pj - 64)*slope      prev tile: (pj - 192)*slope
so it folds into the scalar-engine activation's per-partition bias operand and
the whole score->prob step is one Exp activation per tile (plus a causal
affine_select on diagonal tiles).

Per-core pipeline (all layouts chosen so no transpose of S is ever needed):
  x^T via PE transpose; q^T = wq^T x^T, k^T = wk^T x^T (h on partitions);
  v = x^T-tiles @ wv ([j, h] layout, with a ones column for the denominator);
  S^T[j, i] = k^T-tile.T @ q^T  (one N=256 fp32r matmul per key tile);
  P = exp(S^T + bias) in bf16;  U[i, 65] = sum_kt P^T V_ext (PSUM accumulate);
  out = U[:, :64] * reciprocal(U[:, 64]).
"""

import numpy as np
from contextlib import ExitStack

from concourse import bacc, mybir, tile
from concourse.bass_utils import run_bass_kernel_spmd

B, T, C, HS = 4, 4096, 64, 64
SLOPE = float((2.0**8) ** (-1.0 / 16.0))
NQ = 16               # query tiles of 128 per core
NT = NQ + 1           # key tiles per core (one extra "prev" tile below)
TLOC = NQ * 128       # 2048 queries per core
XROWS = NT * 128      # 2176 x rows per core
NCORES = 8

F32 = mybir.dt.float32
F32R = mybir.dt.float32r
BF16 = mybir.dt.bfloat16

_CACHE: dict = {}


def _build(loop_n=None, stop_after=None):
    nc = bacc.Bacc("TRN2", target_bir_lowering=False, debug=False)

    xs_d = nc.dram_tensor("xs", [XROWS, C], F32, kind="ExternalInput").ap()
    wq_d = nc.dram_tensor("wq", [C, HS], F32, kind="ExternalInput").ap()
    wk_d = nc.dram_tensor("wk", [C, HS], F32, kind="ExternalInput").ap()
    wv_d = nc.dram_tensor("wv", [C, HS], F32, kind="ExternalInput").ap()
    bd_d = nc.dram_tensor("bias_diag", [128, 1], F32, kind="ExternalInput").ap()
    bp_d = nc.dram_tensor("bias_prev", [128, 1], F32, kind="ExternalInput").ap()
    bp0_d = nc.dram_tensor("bias_prev0", [128, 1], F32, kind="ExternalInput").ap()
    id_d = nc.dram_tensor("ident", [128, 128], F32, kind="ExternalInput").ap()
    out_d = nc.dram_tensor("out", [TLOC, HS], F32, kind="ExternalOutput").ap()

    exp_f = mybir.ActivationFunctionType.Exp

    with tile.TileContext(nc) as tc:
        with (
            tc.tile_pool(name="const", bufs=1) as cpool,
            tc.tile_pool(name="big", bufs=1) as bigp,
            tc.tile_pool(name="psum", bufs=2, space="PSUM") as psp,
            tc.tile_pool(name="outp", bufs=3) as outp,
            tc.tile_pool(name="smallp", bufs=3) as smallp,
            ExitStack() as loop_ctx,
        ):
            if loop_n is not None:
                loop_ctx.enter_context(tc.For_i(0, loop_n, 1))
            wq_raw = cpool.tile([C, HS], F32, name="wq_raw")
            wk_raw = cpool.tile([C, HS], F32, name="wk_raw")
            wv_raw = cpool.tile([C, HS], F32, name="wv_raw")
            wq = cpool.tile([C, HS], F32R, name="wq_s")
            wk = cpool.tile([C, HS], F32R, name="wk_s")
            wv = cpool.tile([C, HS], F32R, name="wv_s")
            bd = cpool.tile([128, 1], F32, name="bd_s")
            bp = cpool.tile([128, 1], F32, name="bp_s")
            bp0 = cpool.tile([128, 1], F32, name="bp0_s")
            ident = cpool.tile([128, 128], F32, name="id_s")
            nc.sync.dma_start(wq_raw[:], wq_d)
            nc.sync.dma_start(wk_raw[:], wk_d)
            nc.sync.dma_start(wv_raw[:], wv_d)
            nc.vector.tensor_copy(wq[:], wq_raw[:])
            nc.vector.tensor_copy(wk[:], wk_raw[:])
            nc.vector.tensor_copy(wv[:], wv_raw[:])
            nc.sync.dma_start(bd[:], bd_d)
            nc.sync.dma_start(bp[:], bp_d)
            nc.sync.dma_start(bp0[:], bp0_d)
            nc.sync.dma_start(ident[:], id_d)

            x_all = bigp.tile([128, NT, C], F32, name="x_all")
            nc.sync.dma_start(x_all[:], xs_d.rearrange("(n p) c -> p n c", p=128))

            xT = bigp.tile([C, XROWS], F32R, name="xT")
            kT = bigp.tile([C, XROWS], F32R, name="kT")
            qT = bigp.tile([C, TLOC], F32R, name="qT")
            v_all = bigp.tile([128, NT, 66], BF16, name="v_all")
            p_all = bigp.tile([128, NT, 256], BF16, name="p_all")

            # x^T tiles (c on partitions) via PE transpose
            if stop_after == "dma":
                return _finish(nc)
            for kt in range(NT):
                ps = psp.tile([C, 128], F32, tag="proj", name=f"ps_t{kt}")
                nc.tensor.transpose(ps[:], x_all[:, kt, :], ident[:])
                nc.vector.tensor_copy(xT[:, kt * 128 : (kt + 1) * 128], ps[:])

            # k^T = wk^T @ x^T over all 2176 rows
            off = 0
            while off < XROWS:
                n = min(512, XROWS - off)
                ps = psp.tile([C, 512], F32, tag="proj", name=f"ps_k{off}")
                nc.tensor.matmul(
                    ps[:, :n],
                    wk[:],
                    xT[:, off : off + n],
                    start=True,
                    stop=True,
                )
                nc.scalar.copy(kT[:, off : off + n], ps[:, :n])
                off += n

            # q^T (scaled wq) over the core's 2048 queries (x rows 128..2176)
            for blk in range(4):
                ps = psp.tile([C, 512], F32, tag="proj", name=f"ps_q{blk}")
                nc.tensor.matmul(
                    ps[:],
                    wq[:],
                    xT[:, 128 + blk * 512 : 128 + (blk + 1) * 512],
                    start=True,
                    stop=True,
                )
                nc.vector.tensor_copy(qT[:, blk * 512 : (blk + 1) * 512], ps[:])

            # v tiles in [j, h] layout + ones column for the denominator
            nc.vector.memset(v_all[:, :, 64:65], 1.0)
            for kt in range(NT):
                ps = psp.tile([128, 65], F32, tag="u", name=f"ps_v{kt}")
                nc.tensor.matmul(
                    ps[:, 0:64],
                    xT[:, kt * 128 : (kt + 1) * 128],
                    wv[:],
                    start=True,
                    stop=True,
                )
                nc.scalar.copy(v_all[:, kt, 0:64], ps[:, 0:64])

            if stop_after == "prep":
                return _finish(nc)
            # scores + exp per key tile: S^T[j, i] over the <=256 queries that
            # attend to this tile; left half = diag (query tile kt-1), right
            # half = prev (query tile kt)
            for kt in range(NT):
                lo = 0 if kt >= 1 else 128
                hi = 256 if kt <= NQ - 1 else 128
                qlo = (kt - 1) * 128 + lo
                ps = psp.tile([128, 256], F32, tag="s", name=f"ps_s{kt}")
                nc.tensor.matmul(
                    ps[:, lo:hi],
                    kT[:, kt * 128 : (kt + 1) * 128],
                    qT[:, qlo : qlo + hi - lo],
                    start=True,
                    stop=True,
                )
                if kt >= 1:
                    nc.scalar.activation(
                        p_all[:, kt, 0:128], ps[:, 0:128], exp_f, bias=bd[:, 0:1]
                    )
                    # causal: keep pj <= fi, else 0
                    nc.gpsimd.affine_select(
                        p_all[:, kt, 0:128],
                        p_all[:, kt, 0:128],
                        pattern=[[1, 128]],
                        compare_op=mybir.AluOpType.is_ge,
                        fill=0.0,
                        base=0,
                        channel_multiplier=-1,
                    )
                if kt <= NQ - 1:
                    bias = bp0 if kt == 0 else bp
                    nc.scalar.activation(
                        p_all[:, kt, 128:256], ps[:, 128:256], exp_f, bias=bias[:, 0:1]
                    )

            if stop_after == "scores":
                return _finish(nc)
            # U = P^T @ [V | 1]; out = U[:, :64] / U[:, 64]
            for it in range(NQ):
                ps = psp.tile([128, 65], F32, tag="u", name=f"ps_u{it}")
                nc.tensor.matmul(
                    ps[:], p_all[:, it, 128:256], v_all[:, it, 0:65],
                    start=True, stop=False,
                )
                nc.tensor.matmul(
                    ps[:], p_all[:, it + 1, 0:128], v_all[:, it + 1, 0:65],
                    start=False, stop=True,
                )
                rec = smallp.tile([128, 1], F32, tag="rec", name=f"rec{it}")
                nc.vector.reciprocal(rec[:], ps[:, 64:65])
                ot = outp.tile([128, HS], F32, tag="ot", name=f"ot{it}")
                nc.vector.tensor_scalar_mul(ot[:], ps[:, 0:64], rec[:])
                nc.sync.dma_start(out_d[it * 128 : (it + 1) * 128, :], ot[:])

    nc.compile()
    return nc


def _get_nc(loop_n=None):
    key = ("nc", loop_n)
    if key not in _CACHE:
        _CACHE[key] = _build(loop_n)
    return _CACHE[key]


def make_in_maps(x, Wq, Wk, Wv):
    x = np.ascontiguousarray(np.asarray(x, dtype=np.float32))
    wq_s = np.ascontiguousarray(np.asarray(Wq, dtype=np.float32) * (C**-0.5))
    wk = np.ascontiguousarray(np.asarray(Wk, dtype=np.float32))
    wv = np.ascontiguousarray(np.asarray(Wv, dtype=np.float32))
    pj = np.arange(128, dtype=np.float32)[:, None]
    bias_diag = (pj - 64.0) * SLOPE
    bias_prev = (pj - 192.0) * SLOPE
    bias_neg = np.full((128, 1), -1e30, dtype=np.float32)
    ident = np.eye(128, dtype=np.float32)
    in_maps = []
    for c in range(NCORES):
        b, h = divmod(c, 2)
        q0 = h * TLOC
        if h == 0:
            xs = np.concatenate(
                [np.zeros((128, C), np.float32), x[b, 0:TLOC]], axis=0
            )
            bp0 = bias_neg
        else:
            xs = x[b, q0 - 128 : q0 + TLOC]
            bp0 = bias_prev
        in_maps.append(
            {
                "xs": np.ascontiguousarray(xs),
                "wq": wq_s,
                "wk": wk,
                "wv": wv,
                "bias_diag": np.ascontiguousarray(bias_diag),
                "bias_prev": np.ascontiguousarray(bias_prev),
                "bias_prev0": np.ascontiguousarray(bp0),
                "ident": ident,
            }
        )
    return in_maps


def assemble(results):
    out = np.empty((B, T, C), dtype=np.float32)
    for c in range(NCORES):
        b, h = divmod(c, 2)
        out[b, h * TLOC : (h + 1) * TLOC] = results[c]["out"]
    return out


def run(x, Wq, Wk, Wv, trace=False, loop_n=None):
    nc = _get_nc(loop_n)
    in_maps = make_in_maps(x, Wq, Wk, Wv)
    res = run_bass_kernel_spmd(nc, in_maps, core_ids=list(range(NCORES)), trace=trace)
    return assemble(res.results), res


def kernel(x, Wq, Wk, Wv):
    out, _ = run(x, Wq, Wk, Wv, trace=False)
    return out



# revision 5
# speedup vs baseline: 3.4262x; 3.4262x over previous
"""Single-head causal attention with ALiBi (B=4, T=4096, C=HS=64) on 8 TRN2 cores.

Math: out = softmax(mask((x Wq)(x Wk)^T * C^-0.5 + (j-i)*slope)) @ (x Wv)

ALiBi slope 2^-0.5 makes the softmax an effective 256-wide sliding window
(weights underflow beyond ~128 steps), so each 128-query tile only attends its
own key tile (diag) and the previous one (prev): O(T*256) work.

Design (v2):
- Host uploads x^T directly ([64, 2176] fp16) -- no on-chip transpose.
- G = Wq Wk^T / 8 is folded on host, so QK^T needs ONE projection
  z^T = G^T x^T; scores S = x_tile^T @ z (PE, fp16, shared ldweights with the
  V projection).
- ALiBi bias (pj - 64 - 128*[prev])*slope is split: the per-key part
  e^{(pj-64)*slope} is folded into V's rows (and the denominator ones-column),
  the diag/prev split becomes constant exp biases (+20 / +20-128*slope) that
  cancel per-query in the softmax.  Each exp is a bias-uniform [128, 512]
  activation over 4 score tiles.
- Causal mask: affine_select on Pool over diag P tiles, post-exp.
- U = [P_d^T V_d(q+1)] + [P_p^T V_d(q)] accumulated in PSUM with a ones-column
  denominator; normalize = reciprocal (DVE) + per-partition-scale copies
  (split ACT/DVE).  Output DMA per 4-tile batch.

Sharding: 8 cores = (batch b in 0..3) x (half h in 0..1); core handles 2048
queries, receives x rows [q0-128, q0+2048) zero-padded below row 0.
"""

import numpy as np
from contextlib import ExitStack

import ml_dtypes

from concourse import bacc, mybir, tile
from concourse.bass_utils import run_bass_kernel_spmd

B, T, C, HS = 4, 4096, 64, 64
SLOPE = float((2.0**8) ** (-1.0 / 16.0))
NQ = 16               # query tiles of 128 per core
NT = NQ + 1           # key tiles per core (one extra "prev" tile below)
TLOC = NQ * 128       # 2048 queries per core
XROWS = NT * 128      # 2176 x rows per core
NCORES = 8

BIAS_D = 20.0
BIAS_P = float(20.0 - 128.0 * SLOPE)

F32 = mybir.dt.float32
F16 = mybir.dt.float16
BF16 = mybir.dt.bfloat16

_CACHE: dict = {}


def _build(loop_n=None):
    nc = bacc.Bacc("TRN2", target_bir_lowering=False, debug=False)

    xt_d = nc.dram_tensor("xt", [C, XROWS], F16, kind="ExternalInput").ap()
    g_d = nc.dram_tensor("g", [C, C], F16, kind="ExternalInput").ap()
    wv_d = nc.dram_tensor("wv", [C, HS], F16, kind="ExternalInput").ap()
    ed_d = nc.dram_tensor("ed", [128, 1], F32, kind="ExternalInput").ap()
    edc_d = nc.dram_tensor("edc", [128, NT], BF16, kind="ExternalInput").ap()
    out_d = nc.dram_tensor("out", [TLOC, HS], F32, kind="ExternalOutput").ap()

    exp_f = mybir.ActivationFunctionType.Exp
    copy_f = mybir.ActivationFunctionType.Copy

    with tile.TileContext(nc) as tc:
        with (
            tc.tile_pool(name="const", bufs=1) as cpool,
            tc.tile_pool(name="big", bufs=1) as bigp,
            tc.tile_pool(name="zp", bufs=2, space="PSUM") as zp,
            tc.tile_pool(name="sdp", bufs=2, space="PSUM") as sdp,
            tc.tile_pool(name="spp", bufs=2, space="PSUM") as spp,
            tc.tile_pool(name="vp", bufs=1, space="PSUM") as vp,
            tc.tile_pool(name="up", bufs=1, space="PSUM") as up,
            ExitStack() as loop_ctx,
        ):
            if loop_n is not None:
                loop_ctx.enter_context(tc.For_i(0, loop_n, 1))

            # --- persistent SBUF tiles ---
            g_s = cpool.tile([C, C], F16, name="g_s")
            wv_s = cpool.tile([C, HS], F16, name="wv_s")
            ed_s = cpool.tile([128, 1], F32, name="ed_s")
            dummy = cpool.tile([128, 1], F32, name="dummy")
            bias_d = cpool.tile([128, 1], F32, name="bias_d")
            bias_p = cpool.tile([128, 1], F32, name="bias_p")
            nc.gpsimd.memset(bias_d[:], BIAS_D)
            nc.gpsimd.memset(bias_p[:], BIAS_P)

            xt = bigp.tile([C, XROWS], F16, name="xt_s")
            zt = bigp.tile([C, XROWS], F16, name="zt_s")
            pd = bigp.tile([128, NQ, 128], BF16, name="pd_s")
            pp = bigp.tile([128, NQ, 128], BF16, name="pp_s")
            vd = bigp.tile([128, NT, 66], BF16, name="vd_s")
            outb = bigp.tile([128, NQ, HS], F32, name="outb_s")
            recs = bigp.tile([128, NQ], F32, name="recs_s")

            # Trigger the exp table load on ACT before any real dependency.
            nc.vector.memset(dummy[:], 0.0)
            nc.scalar.activation(dummy[:], dummy[:], exp_f)

            nc.sync.dma_start(g_s[:], g_d)
            nc.sync.dma_start(wv_s[:], wv_d)
            nc.sync.dma_start(ed_s[:], ed_d)
            nc.sync.dma_start(vd[:, :, 64:65], edc_d)
            half = XROWS // 2  # 1088
            nc.sync.dma_start(xt[:, 0:half], xt_d[:, 0:half])
            nc.sync.dma_start(xt[:, half:XROWS], xt_d[:, half:XROWS])

            # --- z projection chunks: z^T = G^T x^T, 512 cols at a time ---
            ZCH = [(c * 512, min(512, XROWS - c * 512)) for c in range(5)]

            def z_chunk(c):
                off, n = ZCH[c]
                zps = zp.tile([C, 512], F32, tag="z", name=f"zps{c}")
                nc.tensor.matmul(
                    zps[:, 0:n], g_s[:], xt[:, off : off + n], start=True, stop=True
                )
                nc.vector.tensor_copy(zt[:, off : off + n], zps[:, 0:n])

            z_chunk(0)
            z_chunk(1)

            # v(0) separately so later V batches align with U batches.
            vps0 = vp.tile([128, 4, HS], F32, tag="v", name="vps0")
            nc.tensor.matmul(
                vps0[:, 0, :], xt[:, 0:128], wv_s[:], start=True, stop=True
            )
            nc.vector.tensor_scalar_mul(vd[:, 0, 0:64], vps0[:, 0, :], ed_s[:, 0:1])

            sd_t = sp_t = vp_t = None
            for a in range(4):
                # S + V matmuls for this macro-batch:
                #   diag key tiles 4a+1..4a+4, prev key tiles 4a..4a+3,
                #   v tiles 4a+1..4a+4.
                sd_t = sdp.tile([128, 4, 128], F32, tag="sd", name=f"sd{a}")
                sp_t = spp.tile([128, 4, 128], F32, tag="sp", name=f"sp{a}")
                vp_t = vp.tile([128, 4, HS], F32, tag="v", name=f"vps{a+1}")
                for m in range(4):
                    kt = 4 * a + m
                    xtile = xt[:, kt * 128 : (kt + 1) * 128]
                    # prev: queries qtile kt vs key tile kt
                    nc.tensor.matmul(
                        sp_t[:, m, :],
                        xtile,
                        zt[:, kt * 128 + 128 : kt * 128 + 256],
                        start=True,
                        stop=True,
                    )
                    kt1 = kt + 1
                    xtile1 = xt[:, kt1 * 128 : (kt1 + 1) * 128]
                    # diag: queries qtile kt1-1=kt vs key tile kt1
                    nc.tensor.matmul(
                        sd_t[:, m, :],
                        xtile1,
                        zt[:, kt1 * 128 : kt1 * 128 + 128],
                        start=True,
                        stop=True,
                    )
                    # v for key tile kt1 (shares ldweights with the diag matmul)
                    nc.tensor.matmul(
                        vp_t[:, m, :], xtile1, wv_s[:], start=True, stop=True
                    )
                # z chunk prefetch for the next macro-batch
                if a + 2 < 5:
                    z_chunk(a + 2)
                # V scale: 4 tiles at once
                nc.vector.tensor_scalar_mul(
                    vd[:, 4 * a + 1 : 4 * a + 5, 0:64], vp_t[:], ed_s[:, 0:1]
                )
                # exp over the 4-tile score batches (bias cancels per query)
                nc.scalar.activation(
                    pd[:, 4 * a : 4 * a + 4, :], sd_t[:], exp_f, bias=bias_d[:, 0:1]
                )
                nc.scalar.activation(
                    pp[:, 4 * a : 4 * a + 4, :], sp_t[:], exp_f, bias=bias_p[:, 0:1]
                )
                # causal mask on diag tiles: keep pj <= fi
                for m in range(4):
                    q = 4 * a + m
                    nc.gpsimd.affine_select(
                        pd[:, q, :],
                        pd[:, q, :],
                        pattern=[[1, 128]],
                        compare_op=mybir.AluOpType.is_ge,
                        fill=0.0,
                        base=0,
                        channel_multiplier=-1,
                    )
                # U accumulation for qtiles 4a..4a+3
                up_t = up.tile([128, 4, 65], F32, tag="u", name=f"u{a}")
                for m in range(4):
                    q = 4 * a + m
                    nc.tensor.matmul(
                        up_t[:, m, :], pd[:, q, :], vd[:, q + 1, 0:65],
                        start=True, stop=False,
                    )
                    nc.tensor.matmul(
                        up_t[:, m, :], pp[:, q, :], vd[:, q, 0:65],
                        start=False, stop=True,
                    )
                # normalize: rec = 1/den, out = U * rec
                nc.vector.reciprocal(recs[:, 4 * a : 4 * a + 4], up_t[:, :, 64])
                for m in range(4):
                    q = 4 * a + m
                    if m % 2 == 0:
                        nc.scalar.activation(
                            outb[:, q, :], up_t[:, m, 0:64], copy_f,
                            scale=recs[:, q : q + 1],
                        )
                    else:
                        nc.vector.tensor_scalar_mul(
                            outb[:, q, :], up_t[:, m, 0:64], recs[:, q : q + 1]
                        )
                nc.sync.dma_start(
                    out_d.rearrange("(n p) c -> p n c", p=128)[:, 4 * a : 4 * a + 4, :],
                    outb[:, 4 * a : 4 * a + 4, :],
                )

    nc.compile()
    return nc


def _get_nc(loop_n=None):
    key = ("nc", loop_n)
    if key not in _CACHE:
        _CACHE[key] = _build(loop_n)
    return _CACHE[key]


def make_in_maps(x, Wq, Wk, Wv):
    x = np.asarray(np.asarray(x), dtype=np.float32)
    Wq = np.asarray(np.asarray(Wq), dtype=np.float64)
    Wk = np.asarray(np.asarray(Wk), dtype=np.float64)
    Wv = np.asarray(np.asarray(Wv), dtype=np.float32)
    g = np.ascontiguousarray((Wq @ Wk.T * (C**-0.5)).astype(np.float16))
    wv = np.ascontiguousarray(Wv.astype(np.float16))
    pj = np.arange(128, dtype=np.float64)[:, None]
    ed = np.exp((pj - 64.0) * SLOPE).astype(np.float32)
    edc_base = np.repeat(ed.astype(ml_dtypes.bfloat16), NT, axis=1)
    in_maps = []
    for c in range(NCORES):
        b, h = divmod(c, 2)
        q0 = h * TLOC
        if h == 0:
            xs = np.concatenate(
                [np.zeros((128, C), np.float32), x[b, 0:TLOC]], axis=0
            )
            edc = edc_base.copy()
            edc[:, 0] = 0  # padding keys must not pollute the denominator
        else:
            xs = x[b, q0 - 128 : q0 + TLOC]
            edc = edc_base
        in_maps.append(
            {
                "xt": np.ascontiguousarray(xs.T.astype(np.float16)),
                "g": g,
                "wv": wv,
                "ed": ed,
                "edc": np.ascontiguousarray(edc),
            }
        )
    return in_maps


def assemble(results):
    out = np.empty((B, T, C), dtype=np.float32)
    for c in range(NCORES):
        b, h = divmod(c, 2)
        out[b, h * TLOC : (h + 1) * TLOC] = results[c]["out"]
    return out


def run(x, Wq, Wk, Wv, trace=False, loop_n=None):
    nc = _get_nc(loop_n)
    in_maps = make_in_maps(x, Wq, Wk, Wv)
    res = run_bass_kernel_spmd(nc, in_maps, core_ids=list(range(NCORES)), trace=trace)
    return assemble(res.results), res


def kernel(x, Wq, Wk, Wv):
    out, _ = run(x, Wq, Wk, Wv, trace=False)
    return out


# revision 6
# speedup vs baseline: 3.4430x; 1.0049x over previous
"""Single-head causal attention with ALiBi (B=4, T=4096, C=HS=64) on 8 TRN2 cores.

Math: out = softmax(mask((x Wq)(x Wk)^T * C^-0.5 + (j-i)*slope)) @ (x Wv)

ALiBi slope 2^-0.5 makes the softmax an effective 256-wide sliding window
(weights underflow beyond ~128 steps), so each 128-query tile only attends its
own key tile (diag) and the previous one (prev): O(T*256) work.

Design (v3):
- Host uploads x^T directly ([64, 2176] fp16) -- no on-chip transpose.
- G = Wq Wk^T / 8 is folded on host, so QK^T needs ONE projection
  z^T = G^T x^T (PE); scores S = x_tile^T @ z (PE, fp16).
- ALiBi bias (pj - 64 - 128*[prev])*slope is split: the per-key part
  e^{(pj-64)*slope} is folded into V's rows (and the denominator ones-column),
  the diag/prev split becomes constant exp biases (+20 / +20-128*slope) that
  cancel per-query in the softmax.  Each exp is a bias-uniform [128, 512]
  activation over 4 score tiles (ACT).
- Causal mask: one DVE multiply per 4-tile batch with an uploaded 0/1 mask.
- U = [P_d^T V_d(q+1)] + [P_p^T V_d(q)] accumulated in PSUM with a ones-column
  denominator; normalize = reciprocal + tensor_scalar (DVE).
- Software pipeline: U/normalize/output-DMA for batch a-1 are emitted during
  batch a, so PE never stalls on the exp/mask chain.

Sharding: 8 cores = (batch b in 0..3) x (half h in 0..1); core handles 2048
queries, receives x rows [q0-128, q0+2048) zero-padded below row 0.
"""

import numpy as np
from contextlib import ExitStack

import ml_dtypes

from concourse import bacc, mybir, tile
from concourse.bass_utils import run_bass_kernel_spmd

B, T, C, HS = 4, 4096, 64, 64
SLOPE = float((2.0**8) ** (-1.0 / 16.0))
NQ = 16               # query tiles of 128 per core
NT = NQ + 1           # key tiles per core (one extra "prev" tile below)
TLOC = NQ * 128       # 2048 queries per core
XROWS = NT * 128      # 2176 x rows per core
NCORES = 8

BIAS_D = 20.0
BIAS_P = float(20.0 - 128.0 * SLOPE)

F32 = mybir.dt.float32
F16 = mybir.dt.float16
BF16 = mybir.dt.bfloat16

_CACHE: dict = {}


def _build(loop_n=None):
    nc = bacc.Bacc("TRN2", target_bir_lowering=False, debug=False)

    xt_d = nc.dram_tensor("xt", [C, XROWS], F16, kind="ExternalInput").ap()
    g_d = nc.dram_tensor("g", [C, C], F16, kind="ExternalInput").ap()
    wv_d = nc.dram_tensor("wv", [C, HS], F16, kind="ExternalInput").ap()
    ed_d = nc.dram_tensor("ed", [128, 1], F32, kind="ExternalInput").ap()
    edc_d = nc.dram_tensor("edc", [128, NT], BF16, kind="ExternalInput").ap()
    mask_d = nc.dram_tensor("mask4", [128, 4 * 128], BF16, kind="ExternalInput").ap()
    out_d = nc.dram_tensor("out", [TLOC, HS], F32, kind="ExternalOutput").ap()

    exp_f = mybir.ActivationFunctionType.Exp

    with tile.TileContext(nc) as tc:
        with (
            tc.tile_pool(name="const", bufs=1) as cpool,
            tc.tile_pool(name="big", bufs=1) as bigp,
            tc.tile_pool(name="zp", bufs=1, space="PSUM") as zp,
            tc.tile_pool(name="sdp", bufs=2, space="PSUM") as sdp,
            tc.tile_pool(name="spp", bufs=2, space="PSUM") as spp,
            tc.tile_pool(name="vp", bufs=1, space="PSUM") as vp,
            tc.tile_pool(name="up", bufs=2, space="PSUM") as up,
            ExitStack() as loop_ctx,
        ):
            if loop_n is not None:
                loop_ctx.enter_context(tc.For_i(0, loop_n, 1))

            # --- persistent SBUF tiles ---
            g_s = cpool.tile([C, C], F16, name="g_s")
            wv_s = cpool.tile([C, HS], F16, name="wv_s")
            ed_s = cpool.tile([128, 1], F32, name="ed_s")
            dummy = cpool.tile([128, 1], F32, name="dummy")
            bias_d = cpool.tile([128, 1], F32, name="bias_d")
            bias_p = cpool.tile([128, 1], F32, name="bias_p")
            mask4 = cpool.tile([128, 4, 128], BF16, name="mask4_s")
            nc.gpsimd.memset(bias_d[:], BIAS_D)
            nc.gpsimd.memset(bias_p[:], BIAS_P)

            xt = bigp.tile([C, XROWS], F16, name="xt_s")
            zt = bigp.tile([C, XROWS], F16, name="zt_s")
            pd = bigp.tile([128, NQ, 128], BF16, name="pd_s")
            pp = bigp.tile([128, NQ, 128], BF16, name="pp_s")
            vd = bigp.tile([128, NT, 66], BF16, name="vd_s")
            outb = bigp.tile([128, NQ, HS], F32, name="outb_s")
            recs = bigp.tile([128, NQ], F32, name="recs_s")

            # Trigger the exp table load on ACT before any real dependency.
            nc.vector.memset(dummy[:], 0.0)
            nc.scalar.activation(dummy[:], dummy[:], exp_f)

            nc.sync.dma_start(g_s[:], g_d)
            nc.sync.dma_start(wv_s[:], wv_d)
            nc.sync.dma_start(ed_s[:], ed_d)
            nc.sync.dma_start(vd[:, :, 64:65], edc_d)
            nc.sync.dma_start(mask4[:], mask_d)
            half = XROWS // 2  # 1088
            nc.sync.dma_start(xt[:, 0:half], xt_d[:, 0:half])
            nc.sync.dma_start(xt[:, half:XROWS], xt_d[:, half:XROWS])

            # --- z projection chunks: z^T = G^T x^T, 512 cols at a time ---
            ZCH = [(c * 512, min(512, XROWS - c * 512)) for c in range(5)]

            def z_chunk(c):
                off, n = ZCH[c]
                zps = zp.tile([C, 512], F32, tag="z", name=f"zps{c}")
                nc.tensor.matmul(
                    zps[:, 0:n], g_s[:], xt[:, off : off + n], start=True, stop=True
                )
                nc.scalar.copy(zt[:, off : off + n], zps[:, 0:n])

            def u_norm_dma(b):
                # U accumulation for qtiles 4b..4b+3 (one batch behind S/exp)
                up_t = up.tile([128, 4, 65], F32, tag="u", name=f"u{b}")
                for m in range(4):
                    q = 4 * b + m
                    nc.tensor.matmul(
                        up_t[:, m, :], pd[:, q, :], vd[:, q + 1, 0:65],
                        start=True, stop=False,
                    )
                    nc.tensor.matmul(
                        up_t[:, m, :], pp[:, q, :], vd[:, q, 0:65],
                        start=False, stop=True,
                    )
                nc.vector.reciprocal(recs[:, 4 * b : 4 * b + 4], up_t[:, :, 64])
                for m in range(4):
                    q = 4 * b + m
                    nc.vector.tensor_scalar_mul(
                        outb[:, q, :], up_t[:, m, 0:64], recs[:, q : q + 1]
                    )
                nc.sync.dma_start(
                    out_d.rearrange("(n p) c -> p n c", p=128)[:, 4 * b : 4 * b + 4, :],
                    outb[:, 4 * b : 4 * b + 4, :],
                )

            z_chunk(0)

            # v(0) separately so later V batches align with U batches.
            vps0 = vp.tile([128, 4, HS], F32, tag="v", name="vps0")
            nc.tensor.matmul(
                vps0[:, 0, :], xt[:, 0:128], wv_s[:], start=True, stop=True
            )
            nc.vector.tensor_scalar_mul(vd[:, 0, 0:64], vps0[:, 0, :], ed_s[:, 0:1])

            z_chunk(1)

            for a in range(4):
                # S + V matmuls for this macro-batch:
                #   diag key tiles 4a+1..4a+4, prev key tiles 4a..4a+3,
                #   v tiles 4a+1..4a+4.
                sd_t = sdp.tile([128, 4, 128], F32, tag="sd", name=f"sd{a}")
                sp_t = spp.tile([128, 4, 128], F32, tag="sp", name=f"sp{a}")
                vp_t = vp.tile([128, 4, HS], F32, tag="v", name=f"vps{a+1}")
                for m in range(4):
                    kt = 4 * a + m
                    xtile = xt[:, kt * 128 : (kt + 1) * 128]
                    # prev: queries qtile kt vs key tile kt
                    nc.tensor.matmul(
                        sp_t[:, m, :],
                        xtile,
                        zt[:, kt * 128 + 128 : kt * 128 + 256],
                        start=True,
                        stop=True,
                    )
                    kt1 = kt + 1
                    xtile1 = xt[:, kt1 * 128 : (kt1 + 1) * 128]
                    # diag: queries qtile kt1-1=kt vs key tile kt1
                    nc.tensor.matmul(
                        sd_t[:, m, :],
                        xtile1,
                        zt[:, kt1 * 128 : kt1 * 128 + 128],
                        start=True,
                        stop=True,
                    )
                    # v for key tile kt1 (shares ldweights with the diag matmul)
                    nc.tensor.matmul(
                        vp_t[:, m, :], xtile1, wv_s[:], start=True, stop=True
                    )
                # z chunk prefetch for the next macro-batch
                if a + 2 < 5:
                    z_chunk(a + 2)
                # V scale: 4 tiles at once
                nc.vector.tensor_scalar_mul(
                    vd[:, 4 * a + 1 : 4 * a + 5, 0:64], vp_t[:], ed_s[:, 0:1]
                )
                # exp over the 4-tile score batches (bias cancels per query)
                nc.scalar.activation(
                    pd[:, 4 * a : 4 * a + 4, :], sd_t[:], exp_f, bias=bias_d[:, 0:1]
                )
                nc.scalar.activation(
                    pp[:, 4 * a : 4 * a + 4, :], sp_t[:], exp_f, bias=bias_p[:, 0:1]
                )
                # causal mask on the 4 diag tiles: one DVE multiply
                nc.vector.tensor_mul(
                    pd[:, 4 * a : 4 * a + 4, :],
                    pd[:, 4 * a : 4 * a + 4, :],
                    mask4[:],
                )
                if a >= 1:
                    u_norm_dma(a - 1)
            u_norm_dma(3)

    nc.compile()
    return nc


def _get_nc(loop_n=None):
    key = ("nc", loop_n)
    if key not in _CACHE:
        _CACHE[key] = _build(loop_n)
    return _CACHE[key]


def make_in_maps(x, Wq, Wk, Wv):
    x = np.asarray(np.asarray(x), dtype=np.float32)
    Wq = np.asarray(np.asarray(Wq), dtype=np.float64)
    Wk = np.asarray(np.asarray(Wk), dtype=np.float64)
    Wv = np.asarray(np.asarray(Wv), dtype=np.float32)
    g = np.ascontiguousarray((Wq @ Wk.T * (C**-0.5)).astype(np.float16))
    wv = np.ascontiguousarray(Wv.astype(np.float16))
    pj = np.arange(128, dtype=np.float64)[:, None]
    ed = np.exp((pj - 64.0) * SLOPE).astype(np.float32)
    edc_base = np.repeat(ed.astype(ml_dtypes.bfloat16), NT, axis=1)
    tri = (np.arange(128)[:, None] <= np.arange(128)[None, :]).astype(
        ml_dtypes.bfloat16
    )
    mask4 = np.ascontiguousarray(np.tile(tri, (1, 4)))
    in_maps = []
    for c in range(NCORES):
        b, h = divmod(c, 2)
        q0 = h * TLOC
        if h == 0:
            xs = np.concatenate(
                [np.zeros((128, C), np.float32), x[b, 0:TLOC]], axis=0
            )
            edc = edc_base.copy()
            edc[:, 0] = 0  # padding keys must not pollute the denominator
        else:
            xs = x[b, q0 - 128 : q0 + TLOC]
            edc = edc_base
        in_maps.append(
            {
                "xt": np.ascontiguousarray(xs.T.astype(np.float16)),
                "g": g,
                "wv": wv,
                "ed": ed,
                "edc": np.ascontiguousarray(edc),
                "mask4": mask4,
            }
        )
    return in_maps


def assemble(results):
    out = np.empty((B, T, C), dtype=np.float32)
    for c in range(NCORES):
        b, h = divmod(c, 2)
        out[b, h * TLOC : (h + 1) * TLOC] = results[c]["out"]
    return out


def run(x, Wq, Wk, Wv, trace=False, loop_n=None):
    nc = _get_nc(loop_n)
    in_maps = make_in_maps(x, Wq, Wk, Wv)
    res = run_bass_kernel_spmd(nc, in_maps, core_ids=list(range(NCORES)), trace=trace)
    return assemble(res.results), res


def kernel(x, Wq, Wk, Wv):
    out, _ = run(x, Wq, Wk, Wv, trace=False)
    return out
